# revision 15
# baseline (speedup 1.0000x reference)
"""Trainium2 Bass kernel for nn_CandidateAwareModel.

Computation (B=2, S=128, H=768, C=5):
    h_p = seq @ W_prd.T + b_prd                        (B,S,H)
    h_a = (seq @ W_arg.T + b_arg).reshape(B,S,C,H)     (B,S,C,H)
    hh  = tanh(h_p[:,p,None,None,:] + h_a[:,None,a,c,:])   (B,Sp,Sa,C,H)
    logits[b,p,c,a] = sum_h hh * W_out[c,h]
    output = logits + (~mask)*NEG ; loss = masked CE against target

Sharding: 8 cores, core ci handles batch b=ci//4 and predicate rows
p in [32*(ci%4), 32*(ci%4+1)).  Device computes logits; the tiny mask
add + log-softmax + scalar loss epilogue runs on host.

Per-core pipeline (ACT-tanh is the roofline: 15.7M tanh / 153.6G/s ~ 102us):
  - h_pT [128(h), 6(hc), 32(p)] f32 computed first (bf16 matmuls).
  - group-pair-outer loop (gb in 4): for hc in 6: on gb==0 produce
    h_aT chunk block (bf16 matmuls + PSUM->SBUF copy folding b_arg),
    then DVE tensor_scalar pre-adds h_p columns (bf16, 4x mode),
    one big ACT Tanh per (gb,hc) batch [128, 5120] -> f32r,
    10 block-diag f32r matmuls accumulate into 2 PSUM banks per gb.
  - per-gb extraction copy + output DMA overlap the next gb.
"""

import numpy as np
import ml_dtypes

import concourse.bass as bass
import concourse.tile as tile
import concourse.mybir as mybir
from concourse import bacc
from concourse import bass_utils

B, S, H, C = 2, 128, 768, 5
NEG = -1024.0
NCORE = 8
PB = S // 4          # 32 predicate rows per core
KC = H // 128        # 6 contraction chunks
HC = H // 128        # 6 h chunks
NJ = C * HC          # 30 (hc,c) chunks
GRP = 4              # p rows per matmul group (N = GRP*128 = 512)
NG = PB // GRP       # 8 groups
GB = 4               # groups per ACT batch / per gb iteration
NGB = NG // GB       # outer gb iterations
PACKN = KC * PB + KC * 128 + KC * S + C * KC * 128  # startup pack cols

F32 = mybir.dt.float32
F32R = mybir.dt.float32r
BF16 = mybir.dt.bfloat16
BF16_NP = ml_dtypes.bfloat16

_CACHE = {}


def _build_program():
    nc = bacc.Bacc("TRN2", debug=False, enable_asserts=True, num_devices=NCORE)

    pack_d = nc.dram_tensor("pack", [128, PACKN], BF16, kind="ExternalInput").ap()
    wprd_d = nc.dram_tensor("wprd", [128, HC, KC, 128], BF16, kind="ExternalInput").ap()
    warg_d = nc.dram_tensor("warg", [128, NJ, KC, 128], BF16, kind="ExternalInput").ap()
    bprd_d = nc.dram_tensor("bprd", [128, HC], F32, kind="ExternalInput").ap()
    barg_d = nc.dram_tensor("barg", [128, NJ], F32, kind="ExternalInput").ap()
    wbd_d = nc.dram_tensor("wbd", [128, NJ, C], BF16, kind="ExternalInput").ap()
    out_d = nc.dram_tensor("out", [C, PB, S], F32, kind="ExternalOutput").ap()

    TANH = mybir.ActivationFunctionType.Tanh

    with tile.TileContext(nc) as tc:
        with tc.tile_pool(name="const", bufs=1) as const, \
             tc.tile_pool(name="wpool", bufs=2) as wpool, \
             tc.tile_pool(name="stg", bufs=3) as stg, \
             tc.tile_pool(name="hhp", bufs=4) as hhp, \
             tc.tile_pool(name="pha", bufs=2, space="PSUM") as pha, \
             tc.tile_pool(name="pred", bufs=1, space="PSUM") as pred:
            # one contiguous startup pack: [seqTp | wprd0 | seqT | w0_0..4]
            # (single 1.2MB DMA instead of 8 small ones on the critical path)
            pack_sb = const.tile([128, PACKN], BF16)
            nc.sync.dma_start(out=pack_sb, in_=pack_d)
            seqTp_sb = pack_sb[:, 0:KC * PB].rearrange("p (k q) -> p k q", k=KC)
            wpsb_0 = pack_sb[:, KC * PB:KC * PB + KC * 128].rearrange(
                "p (k q) -> p k q", k=KC)
            seqT_sb = pack_sb[:, KC * PB + KC * 128:KC * PB + KC * 128 + KC * S
                              ].rearrange("p (k q) -> p k q", k=KC)
            _w0_off = KC * PB + 2 * KC * 128
            wsb0 = [
                pack_sb[:, _w0_off + jj * KC * 128:_w0_off + (jj + 1) * KC * 128
                        ].rearrange("p (k q) -> p k q", k=KC)
                for jj in range(C)
            ]
            bprd_sb = const.tile([128, HC], F32)
            nc.sync.dma_start(out=bprd_sb, in_=bprd_d)
            barg_sb = const.tile([128, NJ], F32)
            nc.sync.dma_start(out=barg_sb, in_=barg_d)
            wbd_sb = const.tile([128, NJ, C], BF16)
            nc.sync.dma_start(out=wbd_sb, in_=wbd_d)

            hpT = const.tile([128, HC, PB], F32)
            haT = const.tile([128, NJ, S], BF16)
            outsb = const.tile([C, PB, S], F32)

            # two PSUM banks, each holding 4 groups at partition offsets 0/32/64/96
            psr = [
                pred.tile([128, GRP, S], F32, name=f"psr{q}", tag=f"psr{q}")
                for q in range(NG // 4)
            ]

            for hc in range(HC):
                # h_pT chunk (b_prd folded in the copy)
                if hc == 0:
                    wpsb = wpsb_0
                else:
                    wpsb = wpool.tile([128, KC, 128], BF16, tag="wprd",
                                      name=f"wpsb{hc}")
                    nc.sync.dma_start(out=wpsb, in_=wprd_d[:, hc, :, :])
                if hc != 0:
                    wsb = wpool.tile([128, C, KC, 128], BF16, tag="warg", bufs=3,
                                     name=f"wsb{hc}")
                    nc.sync.dma_start(
                        out=wsb, in_=warg_d[:, C * hc:C * (hc + 1), :, :])
                ps_hp = pha.tile([128, PB], F32, tag="pshp")
                for k in range(KC):
                    nc.tensor.matmul(
                        ps_hp, wpsb[:, k, :], seqTp_sb[:, k, :],
                        start=(k == 0), stop=(k == KC - 1),
                    )
                nc.vector.tensor_scalar_add(
                    hpT[:, hc, :], ps_hp, bprd_sb[:, hc:hc + 1])

                # h_aT chunks j = 5*hc .. 5*hc+5 (b_arg folded)
                for jj in range(C):
                    j = C * hc + jj
                    wtile = wsb0[jj][:, :, :] if hc == 0 else wsb[:, jj, :, :]
                    ps_ha = pha.tile([128, S], F32, tag="psha")
                    for k in range(KC):
                        nc.tensor.matmul(
                            ps_ha, wtile[:, k, :], seqT_sb[:, k, :],
                            start=(k == 0), stop=(k == KC - 1),
                        )
                    nc.vector.tensor_scalar_add(
                        haT[:, j, :], ps_ha, barg_sb[:, j:j + 1])

                for gb in range(NGB):
                    stage = stg.tile([128, GB, GRP, C, S], BF16, tag="stage")
                    hh = hhp.tile([128, GB, GRP, C, S], BF16, tag="hh")
                    for gs in range(GB):
                        for pp in range(GRP):
                            p = (gb * GB + gs) * GRP + pp
                            nc.vector.tensor_scalar_add(
                                stage[:, gs, pp, :, :],
                                haT[:, C * hc:C * hc + C, :],
                                hpT[:, hc, p:p + 1],
                            )
                    nc.scalar.activation(hh[:], stage[:], TANH)
                    for gs in range(GB):
                        g = gb * GB + gs
                        q, pos = divmod(g, 4)
                        for c in range(C):
                            j = hc * C + c
                            nc.tensor.matmul(
                                psr[q][32 * pos:32 * pos + C, :, :],
                                wbd_sb[:, j, :],
                                hh[:, gs, :, c, :],
                                start=(hc == 0 and c == 0),
                                stop=(hc == HC - 1 and c == C - 1),
                                skip_group_check=True,
                                tile_position=(0, 32 * pos),
                            )
                    if hc == HC - 1:
                        for gs in range(GB):
                            g = gb * GB + gs
                            q, pos = divmod(g, 4)
                            nc.vector.tensor_copy(
                                outsb[:, g * GRP:(g + 1) * GRP, :],
                                psr[q][32 * pos:32 * pos + C, :, :])
                        nc.sync.dma_start(
                            out=out_d[:, gb * GB * GRP:(gb + 1) * GB * GRP, :],
                            in_=outsb[:, gb * GB * GRP:(gb + 1) * GB * GRP, :])

    nc.compile()
    return nc


def _prep_core_inputs(sequence_output, W_prd, b_prd, W_arg, b_arg, W_out):
    seq = np.ascontiguousarray(np.asarray(sequence_output, np.float32))

    # [kin, hc, k, hin] <- W_prd[hc*128+hin, k*128+kin]
    wprd_h = np.ascontiguousarray(
        W_prd.reshape(HC, 128, KC, 128).transpose(3, 0, 2, 1)
    ).astype(BF16_NP)
    # [kin, j=(hc,c), k, chin] <- W_arg[(c,hc,chin), (k,kin)]
    warg_h = np.ascontiguousarray(
        W_arg.reshape(C, HC, 128, KC, 128).transpose(4, 1, 0, 3, 2)
        .reshape(128, NJ, KC, 128)
    ).astype(BF16_NP)
    bprd_h = np.ascontiguousarray(b_prd.reshape(HC, 128).T).astype(np.float32)
    barg_h = np.ascontiguousarray(
        b_arg.reshape(C, HC, 128).transpose(2, 1, 0).reshape(128, NJ)
    ).astype(np.float32)
    wbd_h = np.zeros((128, NJ, C), BF16_NP)
    for hc in range(HC):
        for c in range(C):
            wbd_h[:, hc * C + c, c] = W_out[c, hc * 128:(hc + 1) * 128].astype(BF16_NP)

    in_maps = []
    for ci in range(NCORE):
        b = ci // 4
        pb = ci % 4
        psl = slice(pb * PB, (pb + 1) * PB)
        seqT = seq[b].T  # [kk, a]
        seqT_h = seqT.reshape(KC, 128, S).transpose(1, 0, 2).astype(BF16_NP)
        seqTp_h = (seq[b, psl].T.reshape(KC, 128, PB).transpose(1, 0, 2)
                   ).astype(BF16_NP)
        pack_h = np.concatenate([
            seqTp_h.reshape(128, -1),
            wprd_h[:, 0].reshape(128, -1),
            seqT_h.reshape(128, -1),
            warg_h[:, :C].transpose(0, 1, 2, 3).reshape(128, -1),
        ], axis=1)
        assert pack_h.shape == (128, PACKN)
        in_maps.append(dict(
            pack=np.ascontiguousarray(pack_h), wprd=wprd_h, warg=warg_h,
            bprd=bprd_h, barg=barg_h, wbd=wbd_h,
        ))
    return in_maps


def run_device(inputs, trace=False):
    """Run the bass kernel; returns (full logits (B,S,C,S) f32, BassKernelResults)."""
    if "nc" not in _CACHE:
        _CACHE["nc"] = _build_program()
    nc = _CACHE["nc"]
    in_maps = _prep_core_inputs(
        inputs["sequence_output"], inputs["W_prd"], inputs["b_prd"],
        inputs["W_arg"], inputs["b_arg"], inputs["W_out"])
    res = bass_utils.run_bass_kernel_spmd(
        nc, in_maps, core_ids=list(range(NCORE)), trace=trace)
    logits = np.empty((B, S, C, S), np.float32)
    for ci in range(NCORE):
        b = ci // 4
        pb = ci % 4
        # device out: [c, p_local, a] -> [p_local, c, a]
        logits[b, pb * PB:(pb + 1) * PB] = res.results[ci]["out"].transpose(1, 0, 2)
    return logits, res


def _host_epilogue(logits, inputs):
    attention_mask = np.asarray(inputs["attention_mask"])
    ng_token_mask = np.asarray(inputs["ng_token_mask"])
    target = np.asarray(inputs["target"], np.float32)

    mask = ng_token_mask & (attention_mask[:, None, None, :] > 0)  # (B,S,C,S)
    mask = np.any(mask, axis=2, keepdims=True)                     # (B,S,1,S)
    output = logits + (~mask).astype(np.float32) * NEG             # (B,S,C,S)

    m = output.max(axis=3, keepdims=True)
    lse = m + np.log(np.sum(np.exp(output - m), axis=3, keepdims=True))
    log_sm = output - lse
    loss = np.sum(-log_sm * target) / np.sum(target)
    return np.float32(loss), output.astype(np.float32)


def kernel(sequence_output, attention_mask, ng_token_mask, target,
           W_prd, b_prd, W_arg, b_arg, W_out):
    inputs = dict(
        sequence_output=sequence_output, attention_mask=attention_mask,
        ng_token_mask=ng_token_mask, target=target,
        W_prd=W_prd, b_prd=b_prd, W_arg=W_arg, b_arg=b_arg, W_out=W_out)
    logits, _ = run_device(inputs)
    return _host_epilogue(logits, inputs)


# revision 16
# speedup vs baseline: 1.0185x; 1.0185x over previous
"""Trainium2 Bass kernel for nn_CandidateAwareModel.

Computation (B=2, S=128, H=768, C=5):
    h_p = seq @ W_prd.T + b_prd                        (B,S,H)
    h_a = (seq @ W_arg.T + b_arg).reshape(B,S,C,H)     (B,S,C,H)
    hh  = tanh(h_p[:,p,None,None,:] + h_a[:,None,a,c,:])   (B,Sp,Sa,C,H)
    logits[b,p,c,a] = sum_h hh * W_out[c,h]
    output = logits + (~mask)*NEG ; loss = masked CE against target

Sharding: 8 cores, core ci handles batch b=ci//4 and predicate rows
p in [32*(ci%4), 32*(ci%4+1)).  Device computes logits; the tiny mask
add + log-softmax + scalar loss epilogue runs on host.

Per-core pipeline (ACT-tanh is the roofline: 15.7M tanh / 153.6G/s ~ 102us):
  - h_pT [128(h), 6(hc), 32(p)] f32 computed first (bf16 matmuls).
  - group-pair-outer loop (gb in 4): for hc in 6: on gb==0 produce
    h_aT chunk block (bf16 matmuls + PSUM->SBUF copy folding b_arg),
    then DVE tensor_scalar pre-adds h_p columns (bf16, 4x mode),
    one big ACT Tanh per (gb,hc) batch [128, 5120] -> f32r,
    10 block-diag f32r matmuls accumulate into 2 PSUM banks per gb.
  - per-gb extraction copy + output DMA overlap the next gb.
"""

import numpy as np
import ml_dtypes

import concourse.bass as bass
import concourse.tile as tile
import concourse.mybir as mybir
from concourse import bacc
from concourse import bass_utils

B, S, H, C = 2, 128, 768, 5
NEG = -1024.0
NCORE = 8
PB = S // 4          # 32 predicate rows per core
KC = H // 128        # 6 contraction chunks
HC = H // 128        # 6 h chunks
NJ = C * HC          # 30 (hc,c) chunks
GRP = 4              # p rows per matmul group (N = GRP*128 = 512)
NG = PB // GRP       # 8 groups
GB = 2               # groups per ACT batch / per gb iteration
NGB = NG // GB       # outer gb iterations
PACKN = KC * PB + KC * 128 + KC * S + C * KC * 128  # startup pack cols

F32 = mybir.dt.float32
F32R = mybir.dt.float32r
BF16 = mybir.dt.bfloat16
BF16_NP = ml_dtypes.bfloat16

_CACHE = {}


def _build_program():
    nc = bacc.Bacc("TRN2", debug=False, enable_asserts=True, num_devices=NCORE)

    pack_d = nc.dram_tensor("pack", [128, PACKN], BF16, kind="ExternalInput").ap()
    wprd_d = nc.dram_tensor("wprd", [128, HC, KC, 128], BF16, kind="ExternalInput").ap()
    warg_d = nc.dram_tensor("warg", [128, NJ, KC, 128], BF16, kind="ExternalInput").ap()
    bprd_d = nc.dram_tensor("bprd", [128, HC], F32, kind="ExternalInput").ap()
    barg_d = nc.dram_tensor("barg", [128, NJ], F32, kind="ExternalInput").ap()
    wbd_d = nc.dram_tensor("wbd", [128, NJ, C], BF16, kind="ExternalInput").ap()
    out_d = nc.dram_tensor("out", [C, PB, S], F32, kind="ExternalOutput").ap()

    TANH = mybir.ActivationFunctionType.Tanh

    with tile.TileContext(nc) as tc:
        with tc.tile_pool(name="const", bufs=1) as const, \
             tc.tile_pool(name="wpool", bufs=2) as wpool, \
             tc.tile_pool(name="stg", bufs=3) as stg, \
             tc.tile_pool(name="hhp", bufs=4) as hhp, \
             tc.tile_pool(name="pha", bufs=2, space="PSUM") as pha, \
             tc.tile_pool(name="pred", bufs=1, space="PSUM") as pred:
            # one contiguous startup pack: [seqTp | wprd0 | seqT | w0_0..4]
            # (single 1.2MB DMA instead of 8 small ones on the critical path)
            pack_sb = const.tile([128, PACKN], BF16)
            nc.sync.dma_start(out=pack_sb, in_=pack_d)
            seqTp_sb = pack_sb[:, 0:KC * PB].rearrange("p (k q) -> p k q", k=KC)
            wpsb_0 = pack_sb[:, KC * PB:KC * PB + KC * 128].rearrange(
                "p (k q) -> p k q", k=KC)
            seqT_sb = pack_sb[:, KC * PB + KC * 128:KC * PB + KC * 128 + KC * S
                              ].rearrange("p (k q) -> p k q", k=KC)
            _w0_off = KC * PB + 2 * KC * 128
            wsb0 = [
                pack_sb[:, _w0_off + jj * KC * 128:_w0_off + (jj + 1) * KC * 128
                        ].rearrange("p (k q) -> p k q", k=KC)
                for jj in range(C)
            ]
            bprd_sb = const.tile([128, HC], F32)
            nc.sync.dma_start(out=bprd_sb, in_=bprd_d)
            barg_sb = const.tile([128, NJ], F32)
            nc.sync.dma_start(out=barg_sb, in_=barg_d)
            wbd_sb = const.tile([128, NJ, C], BF16)
            nc.sync.dma_start(out=wbd_sb, in_=wbd_d)

            hpT = const.tile([128, HC, PB], F32)
            haT = const.tile([128, NJ, S], BF16)
            outsb = const.tile([C, PB, S], F32)

            # two PSUM banks, each holding 4 groups at partition offsets 0/32/64/96
            psr = [
                pred.tile([128, GRP, S], F32, name=f"psr{q}", tag=f"psr{q}")
                for q in range(NG // 4)
            ]

            for hc in range(HC):
                # h_pT chunk (b_prd folded in the copy)
                if hc == 0:
                    wpsb = wpsb_0
                else:
                    wpsb = wpool.tile([128, KC, 128], BF16, tag="wprd",
                                      name=f"wpsb{hc}")
                    nc.sync.dma_start(out=wpsb, in_=wprd_d[:, hc, :, :])
                if hc != 0:
                    wsb = wpool.tile([128, C, KC, 128], BF16, tag="warg", bufs=3,
                                     name=f"wsb{hc}")
                    nc.sync.dma_start(
                        out=wsb, in_=warg_d[:, C * hc:C * (hc + 1), :, :])
                ps_hp = pha.tile([128, PB], F32, tag="pshp")
                for k in range(KC):
                    nc.tensor.matmul(
                        ps_hp, wpsb[:, k, :], seqTp_sb[:, k, :],
                        start=(k == 0), stop=(k == KC - 1),
                    )
                nc.vector.tensor_scalar_add(
                    hpT[:, hc, :], ps_hp, bprd_sb[:, hc:hc + 1])

                # h_aT chunks j = 5*hc .. 5*hc+5 (b_arg folded)
                for jj in range(C):
                    j = C * hc + jj
                    wtile = wsb0[jj][:, :, :] if hc == 0 else wsb[:, jj, :, :]
                    ps_ha = pha.tile([128, S], F32, tag="psha")
                    for k in range(KC):
                        nc.tensor.matmul(
                            ps_ha, wtile[:, k, :], seqT_sb[:, k, :],
                            start=(k == 0), stop=(k == KC - 1),
                        )
                    nc.vector.tensor_scalar_add(
                        haT[:, j, :], ps_ha, barg_sb[:, j:j + 1])

                for gb in range(NGB):
                    stage = stg.tile([128, GB, GRP, C, S], BF16, tag="stage")
                    hh = hhp.tile([128, GB, GRP, C, S], BF16, tag="hh")
                    for gs in range(GB):
                        for pp in range(GRP):
                            p = (gb * GB + gs) * GRP + pp
                            nc.vector.tensor_scalar_add(
                                stage[:, gs, pp, :, :],
                                haT[:, C * hc:C * hc + C, :],
                                hpT[:, hc, p:p + 1],
                            )
                    nc.scalar.activation(hh[:], stage[:], TANH)
                    for gs in range(GB):
                        g = gb * GB + gs
                        q, pos = divmod(g, 4)
                        for c in range(C):
                            j = hc * C + c
                            nc.tensor.matmul(
                                psr[q][32 * pos:32 * pos + C, :, :],
                                wbd_sb[:, j, :],
                                hh[:, gs, :, c, :],
                                start=(hc == 0 and c == 0),
                                stop=(hc == HC - 1 and c == C - 1),
                                skip_group_check=True,
                                tile_position=(0, 32 * pos),
                            )
                    if hc == HC - 1:
                        for gs in range(GB):
                            g = gb * GB + gs
                            q, pos = divmod(g, 4)
                            nc.vector.tensor_copy(
                                outsb[:, g * GRP:(g + 1) * GRP, :],
                                psr[q][32 * pos:32 * pos + C, :, :])
                        nc.sync.dma_start(
                            out=out_d[:, gb * GB * GRP:(gb + 1) * GB * GRP, :],
                            in_=outsb[:, gb * GB * GRP:(gb + 1) * GB * GRP, :])

    nc.compile()
    return nc


def _prep_core_inputs(sequence_output, W_prd, b_prd, W_arg, b_arg, W_out):
    seq = np.ascontiguousarray(np.asarray(sequence_output, np.float32))

    # [kin, hc, k, hin] <- W_prd[hc*128+hin, k*128+kin]
    wprd_h = np.ascontiguousarray(
        W_prd.reshape(HC, 128, KC, 128).transpose(3, 0, 2, 1)
    ).astype(BF16_NP)
    # [kin, j=(hc,c), k, chin] <- W_arg[(c,hc,chin), (k,kin)]
    warg_h = np.ascontiguousarray(
        W_arg.reshape(C, HC, 128, KC, 128).transpose(4, 1, 0, 3, 2)
        .reshape(128, NJ, KC, 128)
    ).astype(BF16_NP)
    bprd_h = np.ascontiguousarray(b_prd.reshape(HC, 128).T).astype(np.float32)
    barg_h = np.ascontiguousarray(
        b_arg.reshape(C, HC, 128).transpose(2, 1, 0).reshape(128, NJ)
    ).astype(np.float32)
    wbd_h = np.zeros((128, NJ, C), BF16_NP)
    for hc in range(HC):
        for c in range(C):
            wbd_h[:, hc * C + c, c] = W_out[c, hc * 128:(hc + 1) * 128].astype(BF16_NP)

    in_maps = []
    for ci in range(NCORE):
        b = ci // 4
        pb = ci % 4
        psl = slice(pb * PB, (pb + 1) * PB)
        seqT = seq[b].T  # [kk, a]
        seqT_h = seqT.reshape(KC, 128, S).transpose(1, 0, 2).astype(BF16_NP)
        seqTp_h = (seq[b, psl].T.reshape(KC, 128, PB).transpose(1, 0, 2)
                   ).astype(BF16_NP)
        pack_h = np.concatenate([
            seqTp_h.reshape(128, -1),
            wprd_h[:, 0].reshape(128, -1),
            seqT_h.reshape(128, -1),
            warg_h[:, :C].transpose(0, 1, 2, 3).reshape(128, -1),
        ], axis=1)
        assert pack_h.shape == (128, PACKN)
        in_maps.append(dict(
            pack=np.ascontiguousarray(pack_h), wprd=wprd_h, warg=warg_h,
            bprd=bprd_h, barg=barg_h, wbd=wbd_h,
        ))
    return in_maps


def run_device(inputs, trace=False):
    """Run the bass kernel; returns (full logits (B,S,C,S) f32, BassKernelResults)."""
    if "nc" not in _CACHE:
        _CACHE["nc"] = _build_program()
    nc = _CACHE["nc"]
    in_maps = _prep_core_inputs(
        inputs["sequence_output"], inputs["W_prd"], inputs["b_prd"],
        inputs["W_arg"], inputs["b_arg"], inputs["W_out"])
    res = bass_utils.run_bass_kernel_spmd(
        nc, in_maps, core_ids=list(range(NCORE)), trace=trace)
    logits = np.empty((B, S, C, S), np.float32)
    for ci in range(NCORE):
        b = ci // 4
        pb = ci % 4
        # device out: [c, p_local, a] -> [p_local, c, a]
        logits[b, pb * PB:(pb + 1) * PB] = res.results[ci]["out"].transpose(1, 0, 2)
    return logits, res


def _host_epilogue(logits, inputs):
    attention_mask = np.asarray(inputs["attention_mask"])
    ng_token_mask = np.asarray(inputs["ng_token_mask"])
    target = np.asarray(inputs["target"], np.float32)

    mask = ng_token_mask & (attention_mask[:, None, None, :] > 0)  # (B,S,C,S)
    mask = np.any(mask, axis=2, keepdims=True)                     # (B,S,1,S)
    output = logits + (~mask).astype(np.float32) * NEG             # (B,S,C,S)

    m = output.max(axis=3, keepdims=True)
    lse = m + np.log(np.sum(np.exp(output - m), axis=3, keepdims=True))
    log_sm = output - lse
    loss = np.sum(-log_sm * target) / np.sum(target)
    return np.float32(loss), output.astype(np.float32)


def kernel(sequence_output, attention_mask, ng_token_mask, target,
           W_prd, b_prd, W_arg, b_arg, W_out):
    inputs = dict(
        sequence_output=sequence_output, attention_mask=attention_mask,
        ng_token_mask=ng_token_mask, target=target,
        W_prd=W_prd, b_prd=b_prd, W_arg=W_arg, b_arg=b_arg, W_out=W_out)
    logits, _ = run_device(inputs)
    return _host_epilogue(logits, inputs)


# revision 17
# speedup vs baseline: 1.0238x; 1.0052x over previous
"""Trainium2 Bass kernel for nn_CandidateAwareModel.

Computation (B=2, S=128, H=768, C=5):
    h_p = seq @ W_prd.T + b_prd                        (B,S,H)
    h_a = (seq @ W_arg.T + b_arg).reshape(B,S,C,H)     (B,S,C,H)
    hh  = tanh(h_p[:,p,None,None,:] + h_a[:,None,a,c,:])   (B,Sp,Sa,C,H)
    logits[b,p,c,a] = sum_h hh * W_out[c,h]
    output = logits + (~mask)*NEG ; loss = masked CE against target

Sharding: 8 cores, core ci handles batch b=ci//4 and predicate rows
p in [32*(ci%4), 32*(ci%4+1)).  Device computes logits; the tiny mask
add + log-softmax + scalar loss epilogue runs on host.

Per-core pipeline (ACT-tanh is the roofline: 15.7M tanh / 153.6G/s ~ 102us):
  - h_pT [128(h), 6(hc), 32(p)] f32 computed first (bf16 matmuls).
  - group-pair-outer loop (gb in 4): for hc in 6: on gb==0 produce
    h_aT chunk block (bf16 matmuls + PSUM->SBUF copy folding b_arg),
    then DVE tensor_scalar pre-adds h_p columns (bf16, 4x mode),
    one big ACT Tanh per (gb,hc) batch [128, 5120] -> f32r,
    10 block-diag f32r matmuls accumulate into 2 PSUM banks per gb.
  - per-gb extraction copy + output DMA overlap the next gb.
"""

import numpy as np
import ml_dtypes

import concourse.bass as bass
import concourse.tile as tile
import concourse.mybir as mybir
from concourse import bacc
from concourse import bass_utils

B, S, H, C = 2, 128, 768, 5
NEG = -1024.0
NCORE = 8
PB = S // 4          # 32 predicate rows per core
KC = H // 128        # 6 contraction chunks
HC = H // 128        # 6 h chunks
NJ = C * HC          # 30 (hc,c) chunks
GRP = 4              # p rows per matmul group (N = GRP*128 = 512)
NG = PB // GRP       # 8 groups
GB = 2               # groups per ACT batch / per gb iteration
NGB = NG // GB       # outer gb iterations
PACKN = KC * PB + KC * 128 + KC * S + C * KC * 128  # startup pack cols

F32 = mybir.dt.float32
F32R = mybir.dt.float32r
BF16 = mybir.dt.bfloat16
BF16_NP = ml_dtypes.bfloat16

_CACHE = {}


def _build_program():
    nc = bacc.Bacc("TRN2", debug=False, enable_asserts=True, num_devices=NCORE)

    pack_d = nc.dram_tensor("pack", [128, PACKN], BF16, kind="ExternalInput").ap()
    wprd_d = nc.dram_tensor("wprd", [128, HC, KC, 128], BF16, kind="ExternalInput").ap()
    warg_d = nc.dram_tensor("warg", [128, NJ, KC, 128], BF16, kind="ExternalInput").ap()
    bprd_d = nc.dram_tensor("bprd", [128, HC], F32, kind="ExternalInput").ap()
    barg_d = nc.dram_tensor("barg", [128, NJ], F32, kind="ExternalInput").ap()
    wbd_d = nc.dram_tensor("wbd", [128, NJ, C], BF16, kind="ExternalInput").ap()
    out_d = nc.dram_tensor("out", [C, PB, S], F32, kind="ExternalOutput").ap()

    TANH = mybir.ActivationFunctionType.Tanh

    with tile.TileContext(nc) as tc:
        with tc.tile_pool(name="const", bufs=1) as const, \
             tc.tile_pool(name="wpool", bufs=2) as wpool, \
             tc.tile_pool(name="stg", bufs=3) as stg, \
             tc.tile_pool(name="hhp", bufs=4) as hhp, \
             tc.tile_pool(name="pha", bufs=2, space="PSUM") as pha, \
             tc.tile_pool(name="pred", bufs=1, space="PSUM") as pred:
            # one contiguous startup pack: [seqTp | wprd0 | seqT | w0_0..4]
            # (single 1.2MB DMA instead of 8 small ones on the critical path)
            pack_sb = const.tile([128, PACKN], BF16)
            nc.sync.dma_start(out=pack_sb, in_=pack_d)
            seqTp_sb = pack_sb[:, 0:KC * PB].rearrange("p (k q) -> p k q", k=KC)
            wpsb_0 = pack_sb[:, KC * PB:KC * PB + KC * 128].rearrange(
                "p (k q) -> p k q", k=KC)
            seqT_sb = pack_sb[:, KC * PB + KC * 128:KC * PB + KC * 128 + KC * S
                              ].rearrange("p (k q) -> p k q", k=KC)
            _w0_off = KC * PB + 2 * KC * 128
            wsb0 = [
                pack_sb[:, _w0_off + jj * KC * 128:_w0_off + (jj + 1) * KC * 128
                        ].rearrange("p (k q) -> p k q", k=KC)
                for jj in range(C)
            ]
            bprd_sb = const.tile([128, HC], F32)
            nc.sync.dma_start(out=bprd_sb, in_=bprd_d)
            barg_sb = const.tile([128, NJ], F32)
            nc.sync.dma_start(out=barg_sb, in_=barg_d)
            wbd_sb = const.tile([128, NJ, C], BF16)
            nc.sync.dma_start(out=wbd_sb, in_=wbd_d)

            hpT = const.tile([128, HC, PB], F32)
            haT = const.tile([128, NJ, S], BF16)
            outsb = const.tile([C, PB, S], F32)

            # two PSUM banks, each holding 4 groups at partition offsets 0/32/64/96
            psr = [
                pred.tile([128, GRP, S], F32, name=f"psr{q}", tag=f"psr{q}")
                for q in range(NG // 4)
            ]

            for hc in range(HC):
                # h_pT chunk (b_prd folded in the copy)
                if hc == 0:
                    wpsb = wpsb_0
                else:
                    wpsb = wpool.tile([128, KC, 128], BF16, tag="wprd",
                                      name=f"wpsb{hc}")
                    nc.sync.dma_start(out=wpsb, in_=wprd_d[:, hc, :, :])
                if hc != 0:
                    wsb = wpool.tile([128, C, KC, 128], BF16, tag="warg", bufs=3,
                                     name=f"wsb{hc}")
                    nc.sync.dma_start(
                        out=wsb, in_=warg_d[:, C * hc:C * (hc + 1), :, :])
                ps_hp = pha.tile([128, PB], F32, tag="pshp")
                for k in range(KC):
                    nc.tensor.matmul(
                        ps_hp, wpsb[:, k, :], seqTp_sb[:, k, :],
                        start=(k == 0), stop=(k == KC - 1),
                    )
                nc.vector.tensor_scalar_add(
                    hpT[:, hc, :], ps_hp, bprd_sb[:, hc:hc + 1])

                # h_aT chunks j = 5*hc .. 5*hc+5 (b_arg folded)
                for jj in range(C):
                    j = C * hc + jj
                    wtile = wsb0[jj][:, :, :] if hc == 0 else wsb[:, jj, :, :]
                    ps_ha = pha.tile([128, S], F32, tag="psha")
                    for k in range(KC):
                        nc.tensor.matmul(
                            ps_ha, wtile[:, k, :], seqT_sb[:, k, :],
                            start=(k == 0), stop=(k == KC - 1),
                        )
                    nc.vector.tensor_scalar_add(
                        haT[:, j, :], ps_ha, barg_sb[:, j:j + 1])

                for gb in range(NGB):
                    hh = hhp.tile([128, GB, GRP, C, S], BF16, tag="hh")
                    if hc == 0 and gb == 0:
                        # first batch: fuse the h_p add into the ACT bias so
                        # the kernel's first tanh doesn't wait on the DVE
                        # pre-add chain (slightly more ACT time, much
                        # earlier start)
                        for gs in range(GB):
                            for pp in range(GRP):
                                p = (gb * GB + gs) * GRP + pp
                                nc.scalar.activation(
                                    hh[:, gs, pp, :, :],
                                    haT[:, C * hc:C * hc + C, :],
                                    TANH, bias=hpT[:, hc, p:p + 1])
                    else:
                        stage = stg.tile([128, GB, GRP, C, S], BF16, tag="stage")
                        for gs in range(GB):
                            for pp in range(GRP):
                                p = (gb * GB + gs) * GRP + pp
                                nc.vector.tensor_scalar_add(
                                    stage[:, gs, pp, :, :],
                                    haT[:, C * hc:C * hc + C, :],
                                    hpT[:, hc, p:p + 1],
                                )
                        nc.scalar.activation(hh[:], stage[:], TANH)
                    for gs in range(GB):
                        g = gb * GB + gs
                        q, pos = divmod(g, 4)
                        for c in range(C):
                            j = hc * C + c
                            nc.tensor.matmul(
                                psr[q][32 * pos:32 * pos + C, :, :],
                                wbd_sb[:, j, :],
                                hh[:, gs, :, c, :],
                                start=(hc == 0 and c == 0),
                                stop=(hc == HC - 1 and c == C - 1),
                                skip_group_check=True,
                                tile_position=(0, 32 * pos),
                            )
                    if hc == HC - 1:
                        for gs in range(GB):
                            g = gb * GB + gs
                            q, pos = divmod(g, 4)
                            nc.vector.tensor_copy(
                                outsb[:, g * GRP:(g + 1) * GRP, :],
                                psr[q][32 * pos:32 * pos + C, :, :])
                        nc.sync.dma_start(
                            out=out_d[:, gb * GB * GRP:(gb + 1) * GB * GRP, :],
                            in_=outsb[:, gb * GB * GRP:(gb + 1) * GB * GRP, :])

    nc.compile()
    return nc


def _prep_core_inputs(sequence_output, W_prd, b_prd, W_arg, b_arg, W_out):
    seq = np.ascontiguousarray(np.asarray(sequence_output, np.float32))

    # [kin, hc, k, hin] <- W_prd[hc*128+hin, k*128+kin]
    wprd_h = np.ascontiguousarray(
        W_prd.reshape(HC, 128, KC, 128).transpose(3, 0, 2, 1)
    ).astype(BF16_NP)
    # [kin, j=(hc,c), k, chin] <- W_arg[(c,hc,chin), (k,kin)]
    warg_h = np.ascontiguousarray(
        W_arg.reshape(C, HC, 128, KC, 128).transpose(4, 1, 0, 3, 2)
        .reshape(128, NJ, KC, 128)
    ).astype(BF16_NP)
    bprd_h = np.ascontiguousarray(b_prd.reshape(HC, 128).T).astype(np.float32)
    barg_h = np.ascontiguousarray(
        b_arg.reshape(C, HC, 128).transpose(2, 1, 0).reshape(128, NJ)
    ).astype(np.float32)
    wbd_h = np.zeros((128, NJ, C), BF16_NP)
    for hc in range(HC):
        for c in range(C):
            wbd_h[:, hc * C + c, c] = W_out[c, hc * 128:(hc + 1) * 128].astype(BF16_NP)

    in_maps = []
    for ci in range(NCORE):
        b = ci // 4
        pb = ci % 4
        psl = slice(pb * PB, (pb + 1) * PB)
        seqT = seq[b].T  # [kk, a]
        seqT_h = seqT.reshape(KC, 128, S).transpose(1, 0, 2).astype(BF16_NP)
        seqTp_h = (seq[b, psl].T.reshape(KC, 128, PB).transpose(1, 0, 2)
                   ).astype(BF16_NP)
        pack_h = np.concatenate([
            seqTp_h.reshape(128, -1),
            wprd_h[:, 0].reshape(128, -1),
            seqT_h.reshape(128, -1),
            warg_h[:, :C].transpose(0, 1, 2, 3).reshape(128, -1),
        ], axis=1)
        assert pack_h.shape == (128, PACKN)
        in_maps.append(dict(
            pack=np.ascontiguousarray(pack_h), wprd=wprd_h, warg=warg_h,
            bprd=bprd_h, barg=barg_h, wbd=wbd_h,
        ))
    return in_maps


def run_device(inputs, trace=False):
    """Run the bass kernel; returns (full logits (B,S,C,S) f32, BassKernelResults)."""
    if "nc" not in _CACHE:
        _CACHE["nc"] = _build_program()
    nc = _CACHE["nc"]
    in_maps = _prep_core_inputs(
        inputs["sequence_output"], inputs["W_prd"], inputs["b_prd"],
        inputs["W_arg"], inputs["b_arg"], inputs["W_out"])
    res = bass_utils.run_bass_kernel_spmd(
        nc, in_maps, core_ids=list(range(NCORE)), trace=trace)
    logits = np.empty((B, S, C, S), np.float32)
    for ci in range(NCORE):
        b = ci // 4
        pb = ci % 4
        # device out: [c, p_local, a] -> [p_local, c, a]
        logits[b, pb * PB:(pb + 1) * PB] = res.results[ci]["out"].transpose(1, 0, 2)
    return logits, res


def _host_epilogue(logits, inputs):
    attention_mask = np.asarray(inputs["attention_mask"])
    ng_token_mask = np.asarray(inputs["ng_token_mask"])
    target = np.asarray(inputs["target"], np.float32)

    mask = ng_token_mask & (attention_mask[:, None, None, :] > 0)  # (B,S,C,S)
    mask = np.any(mask, axis=2, keepdims=True)                     # (B,S,1,S)
    output = logits + (~mask).astype(np.float32) * NEG             # (B,S,C,S)

    m = output.max(axis=3, keepdims=True)
    lse = m + np.log(np.sum(np.exp(output - m), axis=3, keepdims=True))
    log_sm = output - lse
    loss = np.sum(-log_sm * target) / np.sum(target)
    return np.float32(loss), output.astype(np.float32)


def kernel(sequence_output, attention_mask, ng_token_mask, target,
           W_prd, b_prd, W_arg, b_arg, W_out):
    inputs = dict(
        sequence_output=sequence_output, attention_mask=attention_mask,
        ng_token_mask=ng_token_mask, target=target,
        W_prd=W_prd, b_prd=b_prd, W_arg=W_arg, b_arg=b_arg, W_out=W_out)
    logits, _ = run_device(inputs)
    return _host_epilogue(logits, inputs)


# revision 18
# speedup vs baseline: 1.0308x; 1.0069x over previous
"""Trainium2 Bass kernel for nn_CandidateAwareModel.

Computation (B=2, S=128, H=768, C=5):
    h_p = seq @ W_prd.T + b_prd                        (B,S,H)
    h_a = (seq @ W_arg.T + b_arg).reshape(B,S,C,H)     (B,S,C,H)
    hh  = tanh(h_p[:,p,None,None,:] + h_a[:,None,a,c,:])   (B,Sp,Sa,C,H)
    logits[b,p,c,a] = sum_h hh * W_out[c,h]
    output = logits + (~mask)*NEG ; loss = masked CE against target

Sharding: 8 cores, core ci handles batch b=ci//4 and predicate rows
p in [32*(ci%4), 32*(ci%4+1)).  Device computes logits; the tiny mask
add + log-softmax + scalar loss epilogue runs on host.

Per-core pipeline (ACT-tanh is the roofline: 15.7M tanh / 153.6G/s ~ 102us):
  - h_pT [128(h), 6(hc), 32(p)] f32 computed first (bf16 matmuls).
  - group-pair-outer loop (gb in 4): for hc in 6: on gb==0 produce
    h_aT chunk block (bf16 matmuls + PSUM->SBUF copy folding b_arg),
    then DVE tensor_scalar pre-adds h_p columns (bf16, 4x mode),
    one big ACT Tanh per (gb,hc) batch [128, 5120] -> f32r,
    10 block-diag f32r matmuls accumulate into 2 PSUM banks per gb.
  - per-gb extraction copy + output DMA overlap the next gb.
"""

import numpy as np
import ml_dtypes

import concourse.bass as bass
import concourse.tile as tile
import concourse.mybir as mybir
from concourse import bacc
from concourse import bass_utils

B, S, H, C = 2, 128, 768, 5
NEG = -1024.0
NCORE = 8
PB = S // 4          # 32 predicate rows per core
KC = H // 128        # 6 contraction chunks
HC = H // 128        # 6 h chunks
NJ = C * HC          # 30 (hc,c) chunks
GRP = 4              # p rows per matmul group (N = GRP*128 = 512)
NG = PB // GRP       # 8 groups
GB = 2               # groups per ACT batch / per gb iteration
NGB = NG // GB       # outer gb iterations
PACKN = KC * PB + KC * 128 + KC * S + C * KC * 128  # startup pack cols

F32 = mybir.dt.float32
F32R = mybir.dt.float32r
BF16 = mybir.dt.bfloat16
BF16_NP = ml_dtypes.bfloat16

_CACHE = {}


def _build_program():
    nc = bacc.Bacc("TRN2", debug=False, enable_asserts=True, num_devices=NCORE)

    pack_d = nc.dram_tensor("pack", [128, PACKN], BF16, kind="ExternalInput").ap()
    wprd_d = nc.dram_tensor("wprd", [128, HC, KC, 128], BF16, kind="ExternalInput").ap()
    warg_d = nc.dram_tensor("warg", [128, NJ, KC, 128], BF16, kind="ExternalInput").ap()
    bprd_d = nc.dram_tensor("bprd", [128, HC], F32, kind="ExternalInput").ap()
    barg_d = nc.dram_tensor("barg", [128, NJ], F32, kind="ExternalInput").ap()
    wbd_d = nc.dram_tensor("wbd", [128, NJ, C], BF16, kind="ExternalInput").ap()
    out_d = nc.dram_tensor("out", [C, PB, S], F32, kind="ExternalOutput").ap()

    TANH = mybir.ActivationFunctionType.Tanh

    with tile.TileContext(nc) as tc:
        with tc.tile_pool(name="const", bufs=1) as const, \
             tc.tile_pool(name="wpool", bufs=2) as wpool, \
             tc.tile_pool(name="stg", bufs=3) as stg, \
             tc.tile_pool(name="hhp", bufs=4) as hhp, \
             tc.tile_pool(name="pha", bufs=2, space="PSUM") as pha, \
             tc.tile_pool(name="pred", bufs=1, space="PSUM") as pred:
            # one contiguous startup pack: [seqTp | wprd0 | seqT | w0_0..4]
            # (single 1.2MB DMA instead of 8 small ones on the critical path)
            pack_sb = const.tile([128, PACKN], BF16)
            nc.sync.dma_start(out=pack_sb, in_=pack_d)
            seqTp_sb = pack_sb[:, 0:KC * PB].rearrange("p (k q) -> p k q", k=KC)
            wpsb_0 = pack_sb[:, KC * PB:KC * PB + KC * 128].rearrange(
                "p (k q) -> p k q", k=KC)
            seqT_sb = pack_sb[:, KC * PB + KC * 128:KC * PB + KC * 128 + KC * S
                              ].rearrange("p (k q) -> p k q", k=KC)
            _w0_off = KC * PB + 2 * KC * 128
            wsb0 = [
                pack_sb[:, _w0_off + jj * KC * 128:_w0_off + (jj + 1) * KC * 128
                        ].rearrange("p (k q) -> p k q", k=KC)
                for jj in range(C)
            ]
            bprd_sb = const.tile([128, HC], F32)
            nc.sync.dma_start(out=bprd_sb, in_=bprd_d)
            barg_sb = const.tile([128, NJ], F32)
            nc.sync.dma_start(out=barg_sb, in_=barg_d)
            wbd_sb = const.tile([128, NJ, C], BF16)
            nc.sync.dma_start(out=wbd_sb, in_=wbd_d)

            hpT = const.tile([128, HC, PB], F32)
            haT = const.tile([128, NJ, S], BF16)
            outsb = const.tile([C, PB, S], F32)

            # two PSUM banks, each holding 4 groups at partition offsets 0/32/64/96
            psr = [
                pred.tile([128, GRP, S], F32, name=f"psr{q}", tag=f"psr{q}")
                for q in range(NG // 4)
            ]

            for hc in range(HC):
                # h_pT chunk (b_prd folded in the copy)
                if hc == 0:
                    wpsb = wpsb_0
                else:
                    wpsb = wpool.tile([128, KC, 128], BF16, tag="wprd",
                                      name=f"wpsb{hc}")
                    nc.sync.dma_start(out=wpsb, in_=wprd_d[:, hc, :, :])
                if hc != 0:
                    wsb = wpool.tile([128, C, KC, 128], BF16, tag="warg", bufs=3,
                                     name=f"wsb{hc}")
                    nc.sync.dma_start(
                        out=wsb, in_=warg_d[:, C * hc:C * (hc + 1), :, :])
                ps_hp = pha.tile([128, PB], F32, tag="pshp")
                for k in range(KC):
                    nc.tensor.matmul(
                        ps_hp, wpsb[:, k, :], seqTp_sb[:, k, :],
                        start=(k == 0), stop=(k == KC - 1),
                    )
                nc.vector.tensor_scalar_add(
                    hpT[:, hc, :], ps_hp, bprd_sb[:, hc:hc + 1])

                # h_aT chunks j = 5*hc .. 5*hc+5 (b_arg folded)
                for jj in range(C):
                    j = C * hc + jj
                    wtile = wsb0[jj][:, :, :] if hc == 0 else wsb[:, jj, :, :]
                    ps_ha = pha.tile([128, S], F32, tag="psha")
                    for k in range(KC):
                        nc.tensor.matmul(
                            ps_ha, wtile[:, k, :], seqT_sb[:, k, :],
                            start=(k == 0), stop=(k == KC - 1),
                        )
                    nc.vector.tensor_scalar_add(
                        haT[:, j, :], ps_ha, barg_sb[:, j:j + 1])

                for gb in range(NGB):
                    hh = hhp.tile([128, GB, GRP, C, S], BF16, tag="hh")
                    if hc == 0 and gb == 0:
                        # first batch: fuse the h_p add into the ACT bias so
                        # the kernel's first tanh doesn't wait on the DVE
                        # pre-add chain (slightly more ACT time, much
                        # earlier start)
                        for gs in range(GB):
                            for pp in range(GRP):
                                p = (gb * GB + gs) * GRP + pp
                                nc.scalar.activation(
                                    hh[:, gs, pp, :, :],
                                    haT[:, C * hc:C * hc + C, :],
                                    TANH, bias=hpT[:, hc, p:p + 1])
                    else:
                        stage = stg.tile([128, GB, GRP, C, S], BF16, tag="stage")
                        for gs in range(GB):
                            for pp in range(GRP):
                                p = (gb * GB + gs) * GRP + pp
                                nc.vector.tensor_scalar_add(
                                    stage[:, gs, pp, :, :],
                                    haT[:, C * hc:C * hc + C, :],
                                    hpT[:, hc, p:p + 1],
                                )
                        if hc == HC - 1 and gb == NGB - 1:
                            # split the kernel's last tanh so the trailing
                            # matmul/extraction chain starts half a batch
                            # earlier
                            for gs in range(GB):
                                nc.scalar.activation(
                                    hh[:, gs], stage[:, gs], TANH)
                        else:
                            nc.scalar.activation(hh[:], stage[:], TANH)
                    for gs in range(GB):
                        g = gb * GB + gs
                        q, pos = divmod(g, 4)
                        for c in range(C):
                            j = hc * C + c
                            nc.tensor.matmul(
                                psr[q][32 * pos:32 * pos + C, :, :],
                                wbd_sb[:, j, :],
                                hh[:, gs, :, c, :],
                                start=(hc == 0 and c == 0),
                                stop=(hc == HC - 1 and c == C - 1),
                                skip_group_check=True,
                                tile_position=(0, 32 * pos),
                            )
                    if hc == HC - 1:
                        for gs in range(GB):
                            g = gb * GB + gs
                            q, pos = divmod(g, 4)
                            nc.vector.tensor_copy(
                                outsb[:, g * GRP:(g + 1) * GRP, :],
                                psr[q][32 * pos:32 * pos + C, :, :])
                        nc.sync.dma_start(
                            out=out_d[:, gb * GB * GRP:(gb + 1) * GB * GRP, :],
                            in_=outsb[:, gb * GB * GRP:(gb + 1) * GB * GRP, :])

    nc.compile()
    return nc


def _prep_core_inputs(sequence_output, W_prd, b_prd, W_arg, b_arg, W_out):
    seq = np.ascontiguousarray(np.asarray(sequence_output, np.float32))

    # [kin, hc, k, hin] <- W_prd[hc*128+hin, k*128+kin]
    wprd_h = np.ascontiguousarray(
        W_prd.reshape(HC, 128, KC, 128).transpose(3, 0, 2, 1)
    ).astype(BF16_NP)
    # [kin, j=(hc,c), k, chin] <- W_arg[(c,hc,chin), (k,kin)]
    warg_h = np.ascontiguousarray(
        W_arg.reshape(C, HC, 128, KC, 128).transpose(4, 1, 0, 3, 2)
        .reshape(128, NJ, KC, 128)
    ).astype(BF16_NP)
    bprd_h = np.ascontiguousarray(b_prd.reshape(HC, 128).T).astype(np.float32)
    barg_h = np.ascontiguousarray(
        b_arg.reshape(C, HC, 128).transpose(2, 1, 0).reshape(128, NJ)
    ).astype(np.float32)
    wbd_h = np.zeros((128, NJ, C), BF16_NP)
    for hc in range(HC):
        for c in range(C):
            wbd_h[:, hc * C + c, c] = W_out[c, hc * 128:(hc + 1) * 128].astype(BF16_NP)

    in_maps = []
    for ci in range(NCORE):
        b = ci // 4
        pb = ci % 4
        psl = slice(pb * PB, (pb + 1) * PB)
        seqT = seq[b].T  # [kk, a]
        seqT_h = seqT.reshape(KC, 128, S).transpose(1, 0, 2).astype(BF16_NP)
        seqTp_h = (seq[b, psl].T.reshape(KC, 128, PB).transpose(1, 0, 2)
                   ).astype(BF16_NP)
        pack_h = np.concatenate([
            seqTp_h.reshape(128, -1),
            wprd_h[:, 0].reshape(128, -1),
            seqT_h.reshape(128, -1),
            warg_h[:, :C].transpose(0, 1, 2, 3).reshape(128, -1),
        ], axis=1)
        assert pack_h.shape == (128, PACKN)
        in_maps.append(dict(
            pack=np.ascontiguousarray(pack_h), wprd=wprd_h, warg=warg_h,
            bprd=bprd_h, barg=barg_h, wbd=wbd_h,
        ))
    return in_maps


def run_device(inputs, trace=False):
    """Run the bass kernel; returns (full logits (B,S,C,S) f32, BassKernelResults)."""
    if "nc" not in _CACHE:
        _CACHE["nc"] = _build_program()
    nc = _CACHE["nc"]
    in_maps = _prep_core_inputs(
        inputs["sequence_output"], inputs["W_prd"], inputs["b_prd"],
        inputs["W_arg"], inputs["b_arg"], inputs["W_out"])
    res = bass_utils.run_bass_kernel_spmd(
        nc, in_maps, core_ids=list(range(NCORE)), trace=trace)
    logits = np.empty((B, S, C, S), np.float32)
    for ci in range(NCORE):
        b = ci // 4
        pb = ci % 4
        # device out: [c, p_local, a] -> [p_local, c, a]
        logits[b, pb * PB:(pb + 1) * PB] = res.results[ci]["out"].transpose(1, 0, 2)
    return logits, res


def _host_epilogue(logits, inputs):
    attention_mask = np.asarray(inputs["attention_mask"])
    ng_token_mask = np.asarray(inputs["ng_token_mask"])
    target = np.asarray(inputs["target"], np.float32)

    mask = ng_token_mask & (attention_mask[:, None, None, :] > 0)  # (B,S,C,S)
    mask = np.any(mask, axis=2, keepdims=True)                     # (B,S,1,S)
    output = logits + (~mask).astype(np.float32) * NEG             # (B,S,C,S)

    m = output.max(axis=3, keepdims=True)
    lse = m + np.log(np.sum(np.exp(output - m), axis=3, keepdims=True))
    log_sm = output - lse
    loss = np.sum(-log_sm * target) / np.sum(target)
    return np.float32(loss), output.astype(np.float32)


def kernel(sequence_output, attention_mask, ng_token_mask, target,
           W_prd, b_prd, W_arg, b_arg, W_out):
    inputs = dict(
        sequence_output=sequence_output, attention_mask=attention_mask,
        ng_token_mask=ng_token_mask, target=target,
        W_prd=W_prd, b_prd=b_prd, W_arg=W_arg, b_arg=b_arg, W_out=W_out)
    logits, _ = run_device(inputs)
    return _host_epilogue(logits, inputs)
